# revision 3
# baseline (speedup 1.0000x reference)
"""RGCN basis-decomposed message passing on 8 TRN2 NeuronCores (v3).

Math: reference computes, per relation r:
    support_r = A @ x[:, :, r]      (A = sparse adjacency from edges, same for all r)
    out = concat_r(support_r) @ W   (W[r] = sum_b w_rel[r,b] * w_bases[b])
By linearity:  out = A @ (sum_r x[:,:,r] @ W_r) = A @ y,   y = x_flat @ w_perm.

Distribution (8 cores):
  - nodes sharded contiguously: core c owns nodes [c*NS, (c+1)*NS)
  - phase A (per core): y_shard = x_shard @ w_perm  (bf16 matmuls on TensorE)
  - AllGather y (f32, 2 chunks) -> every core holds the full y table in DRAM
  - phase C (per core): edges with dst in the shard, bucketed per
    (128-dst group, int16 window), batched into SLABS of S groups. Per slab:
    one dma_gather per window (512B descriptors: elem=128 f32 spanning 2
    consecutive y rows with elem_step=64, so idx = src row directly and only
    the first 64 floats are used -- same DMA time as 256B descs, fewer calls),
    one batched DVE is_equal building all one-hots in transposed layout
    [P, 128 dst, T tiles] (all packed-last bf16 operands -> DVE 2x mode),
    one batched DVE multiply msg = gbuf[:, :, 0:64] * w_bcast, then per-group
    accumulation matmuls lhsT=onehot[:, :, t], rhs=msg[:, t, :].

v2 bottlenecks removed: 98 per-group gather calls (994ns fixed SWDGE cost
each + queue backpressure on GpSimd, 312us busy), 830 per-tile ACT scaled
copies (300us), 830 per-tile DVE one-hot builds (199us). DMA descriptor
drain (~106k descs * 22.75ns / 16 engines ~ 151us) is the remaining wall.
"""

import math
import sys
from contextlib import ExitStack

for _p in ("/opt/trn_rl_repo",):
    if _p not in sys.path:
        sys.path.insert(0, _p)

import ml_dtypes
import numpy as np

import concourse.bacc as bacc
import concourse.bass as bass
import concourse.mybir as mybir
from concourse import library_config
from concourse.bass_utils import run_bass_kernel_spmd

F32 = mybir.dt.float32
BF16 = mybir.dt.bfloat16
I16 = mybir.dt.int16
NPBF16 = ml_dtypes.bfloat16
P = 128
PAD_DST = 200.0  # bf16-exact value outside [0,127]: kills pad slots' one-hots


class Cfg:
    def __init__(self, N, E, D=64, R=8, C=8, S=3):
        self.N, self.E, self.D, self.R, self.C, self.S = N, E, D, R, C, S
        assert N % C == 0
        self.NS = N // C                     # nodes per core
        self.G = math.ceil(self.NS / P)      # 128-node dst groups per core
        self.NS_PAD = self.G * P
        self.NTAB = C * self.NS_PAD          # gather-table rows (all-gathered y)
        # int16 gather windows / AllGather chunks (chunk-major y layout).
        # lo sized so hi fits the 32768-row int16 window limit.
        self.glo = max(1, self.G - 32768 // (C * P)) if self.G > 1 else 1
        if self.G > self.glo:
            self.ag_chunks = [(0, self.glo), (self.glo, self.G)]
        else:
            self.ag_chunks = [(0, self.G)]
        self.NAG = len(self.ag_chunks)
        self.WLO = C * P * self.glo
        assert self.WLO <= 32768
        assert self.NTAB - self.WLO <= 32768
        self.K = D * R // P                  # 128-row contraction chunks
        assert D * R % P == 0
        # slabs of S consecutive dst groups
        self.slabs = [(a, min(self.G, a + S)) for a in range(0, self.G, S)]
        self.NSLAB = len(self.slabs)


def plan_and_pack(cfg, x, edge_src, edge_dst, edge_weight, w_bases, w_rel):
    """Host preprocessing (the sharding step). Returns (plan, in_maps)."""
    C, NS, G, D = cfg.C, cfg.NS, cfg.G, cfg.D
    NS_PAD = cfg.NS_PAD

    es = edge_src.astype(np.int64)
    ed = edge_dst.astype(np.int64)
    ew = edge_weight.astype(np.float32)

    # dense weights: w[r] = sum_b w_rel[r,b] w_bases[b];  w_perm[(i,r), o]
    w = np.einsum("rb,bio->rio", w_rel.astype(np.float64),
                  w_bases.astype(np.float64)).astype(np.float32)
    w_perm = np.ascontiguousarray(w.transpose(1, 0, 2).reshape(D * cfg.R, D))
    wp = np.ascontiguousarray(
        w_perm.reshape(cfg.K, P, D).transpose(1, 0, 2).reshape(P, cfg.K * D)
    ).astype(NPBF16)

    # chunk-major gather-table row for global node v = (c, l):
    # chunk i holds groups [a_i, b_i); within chunk i the row is
    # base_i + (c*P + l%P) * w_i + (l//P - a_i),  w_i = b_i - a_i
    src_c, src_l = es // NS, es % NS
    g_src, p_src = src_l // P, src_l % P
    chunk_of = np.zeros(G, np.int64)
    base = np.zeros(len(cfg.ag_chunks) + 1, np.int64)
    for i, (a, b) in enumerate(cfg.ag_chunks):
        chunk_of[a:b] = i
        base[i + 1] = base[i] + C * P * (b - a)
    ci = chunk_of[g_src]
    a_i = np.array([cfg.ag_chunks[i][0] for i in range(cfg.NAG)])[ci]
    w_i = np.array([b - a for (a, b) in cfg.ag_chunks])[ci]
    trow = base[ci] + (src_c * P + p_src) * w_i + (g_src - a_i)
    dst_c, dst_l = ed // NS, ed % NS
    g_of, n_loc = dst_l // P, dst_l % P
    WLO = cfg.WLO
    is_hi = (trow >= WLO).astype(np.int64)

    # bucket edges by (core, group, window)
    key = (dst_c * G + g_of) * 2 + is_hi
    order = np.argsort(key, kind="stable")
    bounds = np.searchsorted(key[order], np.arange(C * G * 2 + 1))
    cnt = (bounds[1:] - bounds[:-1]).reshape(C, G, 2)

    # tiles per (group, window): max real count across cores (shared program)
    Tj = np.ceil(cnt.max(axis=0) / P).astype(int)          # [G, 2]
    empty = Tj.sum(axis=1) == 0
    Tj[empty, 0] = 1                                       # >=1 tile per group
    T = Tj.sum(axis=1)

    # slab tile layout: per slab: [lo tiles of groups a..b) | hi tiles of a..b)]
    slab_base = np.zeros(cfg.NSLAB + 1, np.int64)
    lo_off = np.zeros(G, np.int64)   # within-slab tile offset of group's lo run
    hi_off = np.zeros(G, np.int64)   # within-slab (after T_lo_s) for hi run
    T_lo_s = np.zeros(cfg.NSLAB, np.int64)
    T_s = np.zeros(cfg.NSLAB, np.int64)
    for si, (a, b) in enumerate(cfg.slabs):
        o = 0
        for g in range(a, b):
            lo_off[g] = o
            o += Tj[g, 0]
        T_lo_s[si] = o
        o2 = 0
        for g in range(a, b):
            hi_off[g] = o2
            o2 += Tj[g, 1]
        T_s[si] = o + o2
        slab_base[si + 1] = slab_base[si] + T_s[si]
    TT = int(slab_base[-1])
    Tmax = int(T_s.max())
    gend = np.array([b for (_, b) in cfg.slabs], np.int64)  # groups done / slab

    # global tile index of each (group, window) run
    slab_of = np.zeros(G, np.int64)
    for si, (a, b) in enumerate(cfg.slabs):
        slab_of[a:b] = si
    run_t0 = np.zeros((G, 2), np.int64)
    for g in range(G):
        si = slab_of[g]
        run_t0[g, 0] = slab_base[si] + lo_off[g]
        run_t0[g, 1] = slab_base[si] + T_lo_s[si] + hi_off[g]

    t_s, n_s, w_s = trow[order], n_loc[order], ew[order]

    iota_rep = np.ascontiguousarray(np.broadcast_to(
        np.arange(P, dtype=np.float32)[None, :, None], (P, P, Tmax)
    )).astype(NPBF16).reshape(P, P * Tmax)

    in_maps = []
    for c in range(C):
        dstloc = np.full((P, TT), PAD_DST, np.float32)
        wgt = np.zeros((P, TT), np.float32)
        idxw = np.zeros((P, 8 * TT), np.int16)
        for g in range(G):
            for j in range(2):
                nt = Tj[g, j]
                if nt == 0:
                    continue
                b0 = (c * G + g) * 2 + j
                lo_, hi_ = bounds[b0], bounds[b0 + 1]
                n = hi_ - lo_
                t0 = run_t0[g, j]
                L = nt * P
                vals = np.zeros(L, np.int64)
                vals[:n] = t_s[lo_:hi_] - (WLO if j else 0)
                # wrapped int16 layout: position i -> [i%16, 8*t0 + i//16],
                # replicated to every 16-partition group
                idxw[:, 8 * t0: 8 * (t0 + nt)] = np.tile(
                    vals.astype(np.int16).reshape(L // 16, 16).T, (8, 1))
                s = np.arange(n)
                pp, tt = s % P, t0 + s // P
                dstloc[pp, tt] = n_s[lo_:hi_]
                wgt[pp, tt] = w_s[lo_:hi_]

        # x^T layout [D*R, NS_PAD] zero-padded, bf16
        xs = x[c * NS:(c + 1) * NS].reshape(NS, D * cfg.R)
        xT = np.zeros((D * cfg.R, NS_PAD), NPBF16)
        xT[:, :NS] = xs.T.astype(NPBF16)
        in_maps.append({
            "xT": xT, "wp": wp.copy(), "gidx": idxw,
            "dstloc": dstloc.astype(NPBF16), "wgt": wgt,
            "iota": iota_rep.copy(),
        })

    plan = {
        "Tj": Tj.tolist(), "TT": TT, "Tmax": Tmax,
        "slab_base": slab_base.tolist(), "T_lo_s": T_lo_s.tolist(),
        "T_s": T_s.tolist(), "lo_off": lo_off.tolist(),
        "hi_off": hi_off.tolist(), "gend": gend.tolist(),
    }
    return plan, in_maps


def build_nc(cfg, plan, nps=6):
    C, G, D, K, S = cfg.C, cfg.G, cfg.D, cfg.K, cfg.S
    NS_PAD, TT, Tmax = cfg.NS_PAD, plan["TT"], plan["Tmax"]
    Tj = plan["Tj"]
    slab_base, T_lo_s, T_s = plan["slab_base"], plan["T_lo_s"], plan["T_s"]
    lo_off, hi_off, gend = plan["lo_off"], plan["hi_off"], plan["gend"]
    NSLAB = cfg.NSLAB

    nc = bacc.Bacc("TRN2", num_swdge_queues=4)

    xT_d = nc.declare_dram_parameter("xT", [K * P, NS_PAD], BF16, isOutput=False)
    wp_d = nc.declare_dram_parameter("wp", [P, K * D], BF16, isOutput=False)
    gidx_d = nc.declare_dram_parameter("gidx", [P, 8 * TT], I16, isOutput=False)
    dstloc_d = nc.declare_dram_parameter("dstloc", [P, TT], BF16, isOutput=False)
    wgt_d = nc.declare_dram_parameter("wgt", [P, TT], F32, isOutput=False)
    iota_d = nc.declare_dram_parameter("iota", [P, P * Tmax], BF16, isOutput=False)
    out_d = nc.declare_dram_parameter("out", [P, G * D], F32, isOutput=True)

    # chunk-major: y_own = concat_i [P, w_i*D]; y_all = concat_i [C*P, w_i*D]
    # +64 f32 tail pad: overlapped 512B gather descs read one row past the end
    y_own = nc.dram_tensor("y_own", [P * G * D], F32)
    y_all = nc.dram_tensor("y_all", [C * P * G * D + 64], F32,
                           addr_space="Shared")

    NCH = min(4, G)  # xT node-range chunks
    NAG = cfg.NAG
    ag_chunks = cfg.ag_chunks
    obase = np.zeros(NAG + 1, np.int64)   # element offsets into y_own
    abase = np.zeros(NAG + 1, np.int64)   # element offsets into y_all
    for i, (a, b) in enumerate(ag_chunks):
        obase[i + 1] = obase[i] + P * (b - a) * D
        abase[i + 1] = abase[i] + C * P * (b - a) * D

    # overlapping-row gather windows: row r covers y_all[r*64 : r*64+128]
    APc = type(y_all[:])

    def win_ap(row0, nrows):
        v = y_all[:]
        return APc(v.tensor, row0 * D, [[D, nrows], [1, 2 * D]])

    with ExitStack() as top:
        sem = top.enter_context
        s_wp = sem(nc.semaphore("s_wp"))
        s_xt = [sem(nc.semaphore(f"s_xt{i}")) for i in range(NCH)]
        s_meta = sem(nc.semaphore("s_meta"))
        s_mmA = sem(nc.semaphore("s_mmA"))
        s_yA = sem(nc.semaphore("s_yA"))
        s_ydma_c = [sem(nc.semaphore(f"s_ydma{i}")) for i in range(NAG)]
        s_cc = sem(nc.semaphore("s_cc"))
        s_g = [sem(nc.semaphore(f"s_g{i}")) for i in range(4)]
        s_v1 = sem(nc.semaphore("s_v1"))
        s_v2 = sem(nc.semaphore("s_v2"))
        s_mm = sem(nc.semaphore("s_mm"))
        s_po = sem(nc.semaphore("s_po"))
        s_od = sem(nc.semaphore("s_od"))

        pa = top
        gidx_sb = pa.enter_context(nc.sbuf_tensor("gidx_sb", [P, 8 * TT], I16))
        dstloc_sb = pa.enter_context(nc.sbuf_tensor("dstloc_sb", [P, TT], BF16))
        wgt_sb = pa.enter_context(nc.sbuf_tensor("wgt_sb", [P, TT], F32))
        iota_sb = pa.enter_context(nc.sbuf_tensor("iota_sb", [P, P, Tmax], BF16))
        out_sb = pa.enter_context(nc.sbuf_tensor("out_sb", [P, G, D], F32))
        xT_sb = pa.enter_context(nc.sbuf_tensor("xT_sb", [P, K, NS_PAD], BF16))
        wp_sb = pa.enter_context(nc.sbuf_tensor("wp_sb", [P, K * D], BF16))
        y_sb = pa.enter_context(nc.sbuf_tensor("y_sb", [P, G, D], F32))
        gbuf = [pa.enter_context(nc.sbuf_tensor(f"gbuf{i}", [P, Tmax, 2 * D], F32))
                for i in range(2)]
        mbuf = [pa.enter_context(nc.sbuf_tensor(f"mbuf{i}", [P, P, Tmax], BF16))
                for i in range(2)]
        msg = [pa.enter_context(nc.sbuf_tensor(f"msg{i}", [P, Tmax, D], BF16))
               for i in range(2)]
        psA = [pa.enter_context(nc.psum_tensor(f"psA{i}", [P, D], F32))
               for i in range(2)]
        ps = [pa.enter_context(nc.psum_tensor(f"psC{i}", [P, D], F32))
              for i in range(nps)]

        step = (G + NCH - 1) // NCH
        nt_chunks = [(i * step, min(G, (i + 1) * step)) for i in range(NCH)]
        nt_chunks = [(a, b) for (a, b) in nt_chunks if b > a]

        def phaseA_sync(sync):
            sync.dma_start(out=wp_sb[:], in_=wp_d[:]).then_inc(s_wp, 16)
            for ci, (a, b) in enumerate(nt_chunks):
                sync.dma_start(
                    out=xT_sb[:, :, a * P:b * P],
                    in_=xT_d.rearrange("(k p) n -> p k n", p=P)[:, :, a * P:b * P],
                ).then_inc(s_xt[ci], 16)
            for i, (a, b) in enumerate(ag_chunks):
                sync.wait_ge(s_yA, b)
                sync.dma_start(
                    out=y_own[int(obase[i]):int(obase[i + 1])].rearrange(
                        "(p w) -> p w", p=P),
                    in_=y_sb[:, a:b, :],
                ).then_inc(s_ydma_c[i], 16)
                if i == 0:
                    # metadata loads deferred past the xT/y-own critical path
                    sync.dma_start(out=gidx_sb[:],
                                   in_=gidx_d[:]).then_inc(s_meta, 16)
                    sync.dma_start(out=dstloc_sb[:],
                                   in_=dstloc_d[:]).then_inc(s_meta, 16)
                    sync.dma_start(out=wgt_sb[:],
                                   in_=wgt_d[:]).then_inc(s_meta, 16)
                    sync.dma_start(
                        out=iota_sb[:],
                        in_=iota_d.rearrange("p (j t) -> p j t", j=P),
                    ).then_inc(s_meta, 16)

        def phaseA_tensor(tensor):
            tensor.wait_ge(s_wp, 16)
            for ci, (a, b) in enumerate(nt_chunks):
                tensor.wait_ge(s_xt[ci], 16)
                for nt in range(a, b):
                    if nt >= 2:
                        tensor.wait_ge(s_yA, nt - 1)
                    for k in range(K):
                        mm = tensor.matmul(
                            psA[nt % 2][:],
                            xT_sb[:, k, nt * P:(nt + 1) * P],
                            wp_sb[:, k * D:(k + 1) * D],
                            start=(k == 0), stop=(k == K - 1),
                        )
                    mm.then_inc(s_mmA, 1)

        # ---------------- phase C ----------------
        blockC = pa.enter_context(nc.Block())

        @blockC.gpsimd
        def _(gpsimd):
            gpsimd.load_library(library_config.mlp)
            for i in range(NAG):
                gpsimd.wait_ge(s_ydma_c[i], 16)
                gpsimd.collective_compute(
                    "AllGather",
                    mybir.AluOpType.bypass,
                    replica_groups=[list(range(C))],
                    ins=[y_own[int(obase[i]):int(obase[i + 1])].opt()],
                    outs=[y_all[int(abase[i]):int(abase[i + 1])].opt()],
                ).then_inc(s_cc)

            lo_win = win_ap(0, cfg.WLO)
            hi_win = win_ap(cfg.WLO, cfg.NTAB - cfg.WLO)
            gpsimd.wait_ge(s_meta, 16)

            def gather_run(si, j):
                t_off = 0 if j == 0 else int(T_lo_s[si])
                nt = int(T_lo_s[si]) if j == 0 else int(T_s[si] - T_lo_s[si])
                if nt == 0:
                    return
                n = nt * P
                t0 = slab_base[si] + t_off
                qn = 2 * (si % 2) + j
                gpsimd.dma_gather(
                    gbuf[si % 2][:, t_off:t_off + nt, :],
                    lo_win if j == 0 else hi_win,
                    gidx_sb[:, 8 * t0: 8 * t0 + n // 16],
                    n, n, 2 * D, elem_step=D,
                    single_packet=False, queue_num=qn,
                ).then_inc(s_g[qn], 16)

            # lo(s) lags hi(s-1) so the hi-chunk AllGather hides under lo
            # gathers; gbuf[s%2] reuse gated on msg(s-2) done.
            gpsimd.wait_ge(s_cc, 1)
            gather_run(0, 0)
            for si in range(1, NSLAB):
                if si >= 2:
                    gpsimd.wait_ge(s_v2, si - 1)
                gather_run(si, 0)
                if si == 1:
                    gpsimd.wait_ge(s_cc, NAG)
                gather_run(si - 1, 1)
            gather_run(NSLAB - 1, 1)

        @blockC.vector
        def _(vector):
            # phase A: psum -> y_sb staging
            for nt in range(G):
                vector.wait_ge(s_mmA, nt + 1)
                vector.tensor_copy(
                    out=y_sb[:, nt, :], in_=psA[nt % 2][:]
                ).then_inc(s_yA, 1)
            vector.wait_ge(s_meta, 64)

            def onehot(si):
                nt = int(T_s[si])
                t0 = slab_base[si]
                if si >= 2:
                    vector.wait_ge(s_mm, gend[si - 2])
                vector.tensor_tensor(
                    out=mbuf[si % 2][:, :, 0:nt],
                    in0=dstloc_sb[:, t0:t0 + nt].rearrange(
                        "p (x t) -> p x t", x=1).to_broadcast([P, P, nt]),
                    in1=iota_sb[:, :, 0:nt],
                    op=mybir.AluOpType.is_equal,
                ).then_inc(s_v1, 1)

            def msgpass(si):
                nt = int(T_s[si])
                t0 = slab_base[si]
                nq = si // 2 + 1
                vector.wait_ge(s_g[2 * (si % 2)], 16 * nq)
                vector.wait_ge(s_g[2 * (si % 2) + 1], 16 * nq)
                vector.tensor_tensor(
                    out=msg[si % 2][:, 0:nt, :],
                    in0=gbuf[si % 2][:, 0:nt, 0:D],
                    in1=wgt_sb[:, t0:t0 + nt].to_broadcast([P, nt, D]),
                    op=mybir.AluOpType.mult,
                ).then_inc(s_v2, 1)

            onehot(0)
            for si in range(1, NSLAB):
                onehot(si)
                msgpass(si - 1)
            msgpass(NSLAB - 1)

        @blockC.tensor
        def _(tensor):
            phaseA_tensor(tensor)
            for si, (a, b) in enumerate(cfg.slabs):
                tensor.wait_ge(s_v1, si + 1)
                tensor.wait_ge(s_v2, si + 1)
                for g in range(a, b):
                    if g >= nps:
                        tensor.wait_ge(s_po, g - nps + 1)
                    tiles = (
                        [lo_off[g] + t for t in range(Tj[g][0])]
                        + [T_lo_s[si] + hi_off[g] + t for t in range(Tj[g][1])]
                    )
                    for i, t in enumerate(tiles):
                        mm = tensor.matmul(
                            ps[g % nps][:],
                            mbuf[si % 2][:, :, t],
                            msg[si % 2][:, t, :],
                            start=(i == 0), stop=(i == len(tiles) - 1),
                        )
                    mm.then_inc(s_mm, 1)

        @blockC.scalar
        def _(scalar):
            for g in range(G):
                scalar.wait_ge(s_mm, g + 1)
                scalar.copy(out_sb[:, g, :], ps[g % nps][:]).then_inc(s_po, 1)

        @blockC.sync
        def _(sync):
            phaseA_sync(sync)
            ostep = (G + 7) // 8
            nod = 0
            for a in range(0, G, ostep):
                b = min(G, a + ostep)
                sync.wait_ge(s_po, b)
                sync.dma_start(
                    out=out_d[:, a * D:b * D], in_=out_sb[:, a:b, :]
                ).then_inc(s_od, 16)
                nod += 16
            sync.wait_ge(s_od, nod)

    nc.compile()
    return nc


def _assemble(cfg, plan, outs):
    D, G, NS = cfg.D, cfg.G, cfg.NS
    full = np.empty((cfg.N, D), np.float32)
    for c in range(cfg.C):
        o = outs[c]["out"].reshape(P, G, D).transpose(1, 0, 2).reshape(
            cfg.NS_PAD, D)
        full[c * NS:(c + 1) * NS] = o[:NS]
    return full


def gnn_kernel(x, edge_src, edge_dst, edge_weight, w_bases, w_rel,
               cfg=None, trace=False):
    if cfg is None:
        cfg = Cfg(N=50000, E=800000)
    plan, in_maps = plan_and_pack(cfg, np.asarray(x), np.asarray(edge_src),
                                  np.asarray(edge_dst), np.asarray(edge_weight),
                                  np.asarray(w_bases), np.asarray(w_rel))
    nc = build_nc(cfg, plan)
    res = run_bass_kernel_spmd(nc, in_maps, list(range(cfg.C)), trace=trace)
    return _assemble(cfg, plan, res.results), res


def kernel(x, edge_src, edge_dst, edge_weight, w_bases, w_rel):
    """Full inputs in, full output out. Shards across 8 NeuronCores inside."""
    cfg = Cfg(N=50000, E=800000)
    plan, in_maps = plan_and_pack(cfg, np.asarray(x), np.asarray(edge_src),
                                  np.asarray(edge_dst), np.asarray(edge_weight),
                                  np.asarray(w_bases), np.asarray(w_rel))
    nc = build_nc(cfg, plan)
    res = run_bass_kernel_spmd(nc, in_maps, list(range(cfg.C)))
    return _assemble(cfg, plan, res.results)


# revision 5
# speedup vs baseline: 1.5980x; 1.5980x over previous
"""RGCN basis-decomposed message passing on 8 TRN2 NeuronCores (v5).

Math: reference computes, per relation r:
    support_r = A @ x[:, :, r]      (A = sparse adjacency from edges, same for all r)
    out = concat_r(support_r) @ W   (W[r] = sum_b w_rel[r,b] * w_bases[b])
By linearity:  out = A @ (sum_r x[:,:,r] @ W_r) = A @ y,   y = x_flat @ w_perm.

Distribution (8 cores):
  - nodes sharded contiguously: core c owns nodes [c*NS, (c+1)*NS)
  - phase A (per core): y_shard = x_shard @ w_perm  (bf16 matmuls on TensorE)
  - AllGather y (f32, 2 chunks) -> every core holds the full y table in DRAM
  - phase C (per core): edges with dst in the shard, bucketed per
    (128-dst group, int16 window), batched into SLABS of S groups. Per slab:
    dma_gather sub-calls of ~8 tiles rotating across all 4 SWDGE queues
    (the Q7 descriptor path runs ~2.9ns/desc only with all queues busy;
    single-queue is 3x slower), one batched DVE is_equal building all
    one-hots in transposed layout [P, 128 dst, T tiles] (packed-last bf16
    operands -> DVE 2x mode), one batched DVE multiply msg = gbuf * w_bcast,
    then per-group accumulation matmuls lhsT=onehot[:, :, t], rhs=msg[:, t, :].
  - lo-window gathers (rows in AG chunk 0) lead hi gathers by NG-1 slabs so
    the Pool engine's wait on the chunk-1 AllGather doesn't starve DMA.

The measured per-edge descriptor cost (~2.9ns, Q7-bound) is the wall; DVE
(~150us), PE (~130us) and the AllGather (~75us) all hide under it.
"""

import math
import sys
from contextlib import ExitStack

for _p in ("/opt/trn_rl_repo",):
    if _p not in sys.path:
        sys.path.insert(0, _p)

import ml_dtypes
import numpy as np

import concourse.bacc as bacc
import concourse.bass as bass
import concourse.mybir as mybir
from concourse import library_config
from concourse.bass_utils import run_bass_kernel_spmd

F32 = mybir.dt.float32
BF16 = mybir.dt.bfloat16
I16 = mybir.dt.int16
NPBF16 = ml_dtypes.bfloat16
P = 128
PAD_DST = 200.0  # bf16-exact value outside [0,127]: kills pad slots' one-hots


class Cfg:
    def __init__(self, N, E, D=64, R=8, C=8, S=3, NG=4, TPC=8):
        self.N, self.E, self.D, self.R, self.C, self.S = N, E, D, R, C, S
        self.NG = NG                         # gbuf ring depth (lo-lead + 1)
        self.TPC = TPC                       # tiles per gather sub-call
        assert N % C == 0
        self.NS = N // C                     # nodes per core
        self.G = math.ceil(self.NS / P)      # 128-node dst groups per core
        self.NS_PAD = self.G * P
        self.NTAB = C * self.NS_PAD          # gather-table rows (all-gathered y)
        # int16 gather windows / AllGather chunks (chunk-major y layout).
        self.glo = max(1, self.G - 32768 // (C * P)) if self.G > 1 else 1
        if self.G > self.glo:
            self.ag_chunks = [(0, self.glo), (self.glo, self.G)]
        else:
            self.ag_chunks = [(0, self.G)]
        self.NAG = len(self.ag_chunks)
        self.WLO = C * P * self.glo
        assert self.WLO <= 32768
        assert self.NTAB - self.WLO <= 32768
        self.K = D * R // P                  # 128-row contraction chunks
        assert D * R % P == 0
        self.slabs = [(a, min(self.G, a + S)) for a in range(0, self.G, S)]
        self.NSLAB = len(self.slabs)


def plan_and_pack(cfg, x, edge_src, edge_dst, edge_weight, w_bases, w_rel):
    """Host preprocessing (the sharding step). Returns (plan, in_maps)."""
    C, NS, G, D = cfg.C, cfg.NS, cfg.G, cfg.D
    NS_PAD = cfg.NS_PAD

    es = edge_src.astype(np.int64)
    ed = edge_dst.astype(np.int64)
    ew = edge_weight.astype(np.float32)

    # dense weights: w[r] = sum_b w_rel[r,b] w_bases[b];  w_perm[(i,r), o]
    w = np.einsum("rb,bio->rio", w_rel.astype(np.float64),
                  w_bases.astype(np.float64)).astype(np.float32)
    w_perm = np.ascontiguousarray(w.transpose(1, 0, 2).reshape(D * cfg.R, D))
    wp = np.ascontiguousarray(
        w_perm.reshape(cfg.K, P, D).transpose(1, 0, 2).reshape(P, cfg.K * D)
    ).astype(NPBF16)

    # chunk-major gather-table row for global node v = (c, l):
    # chunk i holds groups [a_i, b_i); within chunk i the row is
    # base_i + (c*P + l%P) * w_i + (l//P - a_i),  w_i = b_i - a_i
    src_c, src_l = es // NS, es % NS
    g_src, p_src = src_l // P, src_l % P
    chunk_of = np.zeros(G, np.int64)
    base = np.zeros(len(cfg.ag_chunks) + 1, np.int64)
    for i, (a, b) in enumerate(cfg.ag_chunks):
        chunk_of[a:b] = i
        base[i + 1] = base[i] + C * P * (b - a)
    ci = chunk_of[g_src]
    a_i = np.array([cfg.ag_chunks[i][0] for i in range(cfg.NAG)])[ci]
    w_i = np.array([b - a for (a, b) in cfg.ag_chunks])[ci]
    trow = base[ci] + (src_c * P + p_src) * w_i + (g_src - a_i)
    dst_c, dst_l = ed // NS, ed % NS
    g_of, n_loc = dst_l // P, dst_l % P
    WLO = cfg.WLO
    is_hi = (trow >= WLO).astype(np.int64)

    # bucket edges by (core, group, window)
    key = (dst_c * G + g_of) * 2 + is_hi
    order = np.argsort(key, kind="stable")
    bounds = np.searchsorted(key[order], np.arange(C * G * 2 + 1))
    cnt = (bounds[1:] - bounds[:-1]).reshape(C, G, 2)

    # tiles per (group, window): max real count across cores (shared program)
    Tj = np.ceil(cnt.max(axis=0) / P).astype(int)          # [G, 2]
    empty = Tj.sum(axis=1) == 0
    Tj[empty, 0] = 1                                       # >=1 tile per group
    T = Tj.sum(axis=1)

    # slab tile layout: per slab: [lo tiles of groups a..b) | hi tiles of a..b)]
    slab_base = np.zeros(cfg.NSLAB + 1, np.int64)
    lo_off = np.zeros(G, np.int64)
    hi_off = np.zeros(G, np.int64)
    T_lo_s = np.zeros(cfg.NSLAB, np.int64)
    T_s = np.zeros(cfg.NSLAB, np.int64)
    for si, (a, b) in enumerate(cfg.slabs):
        o = 0
        for g in range(a, b):
            lo_off[g] = o
            o += Tj[g, 0]
        T_lo_s[si] = o
        o2 = 0
        for g in range(a, b):
            hi_off[g] = o2
            o2 += Tj[g, 1]
        T_s[si] = o + o2
        slab_base[si + 1] = slab_base[si] + T_s[si]
    TT = int(slab_base[-1])
    Tmax = int(T_s.max())
    gend = np.array([b for (_, b) in cfg.slabs], np.int64)

    slab_of = np.zeros(G, np.int64)
    for si, (a, b) in enumerate(cfg.slabs):
        slab_of[a:b] = si
    run_t0 = np.zeros((G, 2), np.int64)
    for g in range(G):
        si = slab_of[g]
        run_t0[g, 0] = slab_base[si] + lo_off[g]
        run_t0[g, 1] = slab_base[si] + T_lo_s[si] + hi_off[g]

    t_s, n_s, w_s = trow[order], n_loc[order], ew[order]

    iota_rep = np.ascontiguousarray(np.broadcast_to(
        np.arange(P, dtype=np.float32)[None, :, None], (P, P, Tmax)
    )).astype(NPBF16).reshape(P, P * Tmax)

    in_maps = []
    for c in range(C):
        dstloc = np.full((P, TT), PAD_DST, np.float32)
        wgt = np.zeros((P, TT), np.float32)
        idxw = np.zeros((P, 8 * TT), np.int16)
        for g in range(G):
            for j in range(2):
                nt = Tj[g, j]
                if nt == 0:
                    continue
                b0 = (c * G + g) * 2 + j
                lo_, hi_ = bounds[b0], bounds[b0 + 1]
                n = hi_ - lo_
                t0 = run_t0[g, j]
                L = nt * P
                vals = np.zeros(L, np.int64)
                vals[:n] = t_s[lo_:hi_] - (WLO if j else 0)
                # wrapped int16 layout: position i -> [i%16, 8*t0 + i//16],
                # replicated to every 16-partition group
                idxw[:, 8 * t0: 8 * (t0 + nt)] = np.tile(
                    vals.astype(np.int16).reshape(L // 16, 16).T, (8, 1))
                s = np.arange(n)
                pp, tt = s % P, t0 + s // P
                dstloc[pp, tt] = n_s[lo_:hi_]
                wgt[pp, tt] = w_s[lo_:hi_]

        # x^T layout [D*R, NS_PAD] zero-padded, bf16
        xs = x[c * NS:(c + 1) * NS].reshape(NS, D * cfg.R)
        xT = np.zeros((D * cfg.R, NS_PAD), NPBF16)
        xT[:, :NS] = xs.T.astype(NPBF16)
        in_maps.append({
            "xT": xT, "wp": wp.copy(), "gidx": idxw,
            "dstloc": dstloc.astype(NPBF16), "wgt": wgt,
            "iota": iota_rep.copy(),
        })

    plan = {
        "Tj": Tj.tolist(), "TT": TT, "Tmax": Tmax,
        "slab_base": slab_base.tolist(), "T_lo_s": T_lo_s.tolist(),
        "T_s": T_s.tolist(), "lo_off": lo_off.tolist(),
        "hi_off": hi_off.tolist(), "gend": gend.tolist(),
    }
    return plan, in_maps


def build_nc(cfg, plan, nps=6):
    C, G, D, K, S = cfg.C, cfg.G, cfg.D, cfg.K, cfg.S
    NS_PAD, TT, Tmax = cfg.NS_PAD, plan["TT"], plan["Tmax"]
    Tj = plan["Tj"]
    slab_base, T_lo_s, T_s = plan["slab_base"], plan["T_lo_s"], plan["T_s"]
    lo_off, hi_off, gend = plan["lo_off"], plan["hi_off"], plan["gend"]
    NSLAB, NG, TPC = cfg.NSLAB, cfg.NG, cfg.TPC
    K_LEAD = NG - 1

    nc = bacc.Bacc("TRN2", num_swdge_queues=4)

    xT_d = nc.declare_dram_parameter("xT", [K * P, NS_PAD], BF16, isOutput=False)
    wp_d = nc.declare_dram_parameter("wp", [P, K * D], BF16, isOutput=False)
    gidx_d = nc.declare_dram_parameter("gidx", [P, 8 * TT], I16, isOutput=False)
    dstloc_d = nc.declare_dram_parameter("dstloc", [P, TT], BF16, isOutput=False)
    wgt_d = nc.declare_dram_parameter("wgt", [P, TT], F32, isOutput=False)
    iota_d = nc.declare_dram_parameter("iota", [P, P * Tmax], BF16, isOutput=False)
    out_d = nc.declare_dram_parameter("out", [P, G * D], F32, isOutput=True)

    # chunk-major: y_own = concat_i [P, w_i*D]; y_all = concat_i [C*P, w_i*D]
    y_own = nc.dram_tensor("y_own", [P * G * D], F32)
    y_all = nc.dram_tensor("y_all", [C * P * G * D], F32, addr_space="Shared")

    NCH = min(4, G)  # xT node-range chunks
    NAG = cfg.NAG
    ag_chunks = cfg.ag_chunks
    obase = np.zeros(NAG + 1, np.int64)
    abase = np.zeros(NAG + 1, np.int64)
    for i, (a, b) in enumerate(ag_chunks):
        obase[i + 1] = obase[i] + P * (b - a) * D
        abase[i + 1] = abase[i] + C * P * (b - a) * D

    # ---- gather sub-call schedule (shared between engines) ----
    # per slab: lo run [0, T_lo_s), hi run [T_lo_s, T_s), chopped into
    # sub-calls of <= TPC tiles; queue = global call counter % 4.
    # call order: lo(s) leads hi(s - K_LEAD).
    def run_calls(si, j):
        t_off = 0 if j == 0 else int(T_lo_s[si])
        nt = int(T_lo_s[si]) if j == 0 else int(T_s[si] - T_lo_s[si])
        out = []
        for a in range(0, nt, TPC):
            out.append((si, t_off + a, min(TPC, nt - a)))
        return out

    call_seq = []        # (si, t_off, nt, qn) in Pool program order
    gate = {}            # call index -> ('cc1'|'cc2'|None)
    qcnt = [0, 0, 0, 0]
    qcount_hi = {}       # si -> per-queue counts after hi(si) fully issued
    first_lo, first_hi = True, True

    def emit(calls, g8):
        nonlocal first_lo, first_hi
        for k, (si, t_off, nt) in enumerate(calls):
            qn = len(call_seq) % 4
            gate[len(call_seq)] = g8 if k == 0 else None
            call_seq.append((si, t_off, nt, qn))
            qcnt[qn] += 1

    for s in range(NSLAB):
        emit(run_calls(s, 0),
             'cc1' if s == 0 else ('gb' if s >= NG else None))
        if s >= K_LEAD:
            emit(run_calls(s - K_LEAD, 1), 'cc2' if first_hi else None)
            first_hi = False
            qcount_hi[s - K_LEAD] = tuple(qcnt)
    for s in range(max(0, NSLAB - K_LEAD), NSLAB):
        emit(run_calls(s, 1), 'cc2' if first_hi else None)
        first_hi = False
        qcount_hi[s] = tuple(qcnt)

    with ExitStack() as top:
        sem = top.enter_context
        s_wp = sem(nc.semaphore("s_wp"))
        s_xt = [sem(nc.semaphore(f"s_xt{i}")) for i in range(NCH)]
        s_meta = sem(nc.semaphore("s_meta"))
        s_mmA = sem(nc.semaphore("s_mmA"))
        s_yA = sem(nc.semaphore("s_yA"))
        s_ydma_c = [sem(nc.semaphore(f"s_ydma{i}")) for i in range(NAG)]
        s_cc = sem(nc.semaphore("s_cc"))
        s_g = [sem(nc.semaphore(f"s_g{i}")) for i in range(4)]
        s_v1 = sem(nc.semaphore("s_v1"))
        s_v2 = sem(nc.semaphore("s_v2"))
        s_mm = sem(nc.semaphore("s_mm"))
        s_po = sem(nc.semaphore("s_po"))
        s_od = sem(nc.semaphore("s_od"))

        pa = top
        gidx_sb = pa.enter_context(nc.sbuf_tensor("gidx_sb", [P, 8 * TT], I16))
        dstloc_sb = pa.enter_context(nc.sbuf_tensor("dstloc_sb", [P, TT], BF16))
        wgt_sb = pa.enter_context(nc.sbuf_tensor("wgt_sb", [P, TT], F32))
        iota_sb = pa.enter_context(nc.sbuf_tensor("iota_sb", [P, P, Tmax], BF16))
        out_sb = pa.enter_context(nc.sbuf_tensor("out_sb", [P, G, D], F32))
        xT_sb = pa.enter_context(nc.sbuf_tensor("xT_sb", [P, K, NS_PAD], BF16))
        wp_sb = pa.enter_context(nc.sbuf_tensor("wp_sb", [P, K * D], BF16))
        y_sb = pa.enter_context(nc.sbuf_tensor("y_sb", [P, G, D], F32))
        gbuf = [pa.enter_context(nc.sbuf_tensor(f"gbuf{i}", [P, Tmax, D], F32))
                for i in range(NG)]
        mbuf = [pa.enter_context(nc.sbuf_tensor(f"mbuf{i}", [P, P, Tmax], BF16))
                for i in range(2)]
        msg = [pa.enter_context(nc.sbuf_tensor(f"msg{i}", [P, Tmax, D], BF16))
               for i in range(2)]
        psA = [pa.enter_context(nc.psum_tensor(f"psA{i}", [P, D], F32))
               for i in range(2)]
        ps = [pa.enter_context(nc.psum_tensor(f"psC{i}", [P, D], F32))
              for i in range(nps)]

        step = (G + NCH - 1) // NCH
        nt_chunks = [(i * step, min(G, (i + 1) * step)) for i in range(NCH)]
        nt_chunks = [(a, b) for (a, b) in nt_chunks if b > a]

        def phaseA_sync(sync):
            sync.dma_start(out=wp_sb[:], in_=wp_d[:]).then_inc(s_wp, 16)
            for ci, (a, b) in enumerate(nt_chunks):
                sync.dma_start(
                    out=xT_sb[:, :, a * P:b * P],
                    in_=xT_d.rearrange("(k p) n -> p k n", p=P)[:, :, a * P:b * P],
                ).then_inc(s_xt[ci], 16)
            for i, (a, b) in enumerate(ag_chunks):
                sync.wait_ge(s_yA, b)
                sync.dma_start(
                    out=y_own[int(obase[i]):int(obase[i + 1])].rearrange(
                        "(p w) -> p w", p=P),
                    in_=y_sb[:, a:b, :],
                ).then_inc(s_ydma_c[i], 16)
                if i == 0:
                    sync.dma_start(out=gidx_sb[:],
                                   in_=gidx_d[:]).then_inc(s_meta, 16)
                    sync.dma_start(out=dstloc_sb[:],
                                   in_=dstloc_d[:]).then_inc(s_meta, 16)
                    sync.dma_start(out=wgt_sb[:],
                                   in_=wgt_d[:]).then_inc(s_meta, 16)
                    sync.dma_start(
                        out=iota_sb[:],
                        in_=iota_d.rearrange("p (j t) -> p j t", j=P),
                    ).then_inc(s_meta, 16)

        def phaseA_tensor(tensor):
            tensor.wait_ge(s_wp, 16)
            for ci, (a, b) in enumerate(nt_chunks):
                tensor.wait_ge(s_xt[ci], 16)
                for nt in range(a, b):
                    if nt >= 2:
                        tensor.wait_ge(s_yA, nt - 1)
                    for k in range(K):
                        mm = tensor.matmul(
                            psA[nt % 2][:],
                            xT_sb[:, k, nt * P:(nt + 1) * P],
                            wp_sb[:, k * D:(k + 1) * D],
                            start=(k == 0), stop=(k == K - 1),
                        )
                    mm.then_inc(s_mmA, 1)

        # ---------------- phase C ----------------
        blockC = pa.enter_context(nc.Block())

        @blockC.gpsimd
        def _(gpsimd):
            gpsimd.load_library(library_config.mlp)
            for i in range(NAG):
                gpsimd.wait_ge(s_ydma_c[i], 16)
                gpsimd.collective_compute(
                    "AllGather",
                    mybir.AluOpType.bypass,
                    replica_groups=[list(range(C))],
                    ins=[y_own[int(obase[i]):int(obase[i + 1])].opt()],
                    outs=[y_all[int(abase[i]):int(abase[i + 1])].opt()],
                ).then_inc(s_cc)

            y_rows = y_all.rearrange("(r d) -> r d", d=D)
            lo_win = y_rows[0:cfg.WLO, :]
            hi_win = y_rows[cfg.WLO:cfg.NTAB, :]
            gpsimd.wait_ge(s_meta, 16)

            for ci, (si, t_off, nt, qn) in enumerate(call_seq):
                g8 = gate[ci]
                if g8 == 'cc1':
                    gpsimd.wait_ge(s_cc, 1)
                elif g8 == 'cc2':
                    gpsimd.wait_ge(s_cc, NAG)
                elif g8 == 'gb':
                    gpsimd.wait_ge(s_v2, si - NG + 1)
                is_hi = t_off >= int(T_lo_s[si])
                n = nt * P
                t0 = slab_base[si] + t_off
                gpsimd.dma_gather(
                    gbuf[si % NG][:, t_off:t_off + nt, :],
                    hi_win if is_hi else lo_win,
                    gidx_sb[:, 8 * t0: 8 * t0 + n // 16],
                    n, n, D,
                    single_packet=False, queue_num=qn,
                ).then_inc(s_g[qn], 16)

        @blockC.vector
        def _(vector):
            for nt in range(G):
                vector.wait_ge(s_mmA, nt + 1)
                vector.tensor_copy(
                    out=y_sb[:, nt, :], in_=psA[nt % 2][:]
                ).then_inc(s_yA, 1)
            vector.wait_ge(s_meta, 64)

            def onehot(si):
                nt = int(T_s[si])
                t0 = slab_base[si]
                if si >= 2:
                    vector.wait_ge(s_mm, gend[si - 2])
                vector.tensor_tensor(
                    out=mbuf[si % 2][:, :, 0:nt],
                    in0=dstloc_sb[:, t0:t0 + nt].rearrange(
                        "p (x t) -> p x t", x=1).to_broadcast([P, P, nt]),
                    in1=iota_sb[:, :, 0:nt],
                    op=mybir.AluOpType.is_equal,
                ).then_inc(s_v1, 1)

            def msgpass(si):
                nt = int(T_s[si])
                t0 = slab_base[si]
                for q in range(4):
                    if qcount_hi[si][q]:
                        vector.wait_ge(s_g[q], 16 * qcount_hi[si][q])
                vector.tensor_tensor(
                    out=msg[si % 2][:, 0:nt, :],
                    in0=gbuf[si % NG][:, 0:nt, :],
                    in1=wgt_sb[:, t0:t0 + nt].to_broadcast([P, nt, D]),
                    op=mybir.AluOpType.mult,
                ).then_inc(s_v2, 1)

            onehot(0)
            for si in range(1, NSLAB):
                onehot(si)
                msgpass(si - 1)
            msgpass(NSLAB - 1)

        @blockC.tensor
        def _(tensor):
            phaseA_tensor(tensor)
            for si, (a, b) in enumerate(cfg.slabs):
                tensor.wait_ge(s_v1, si + 1)
                tensor.wait_ge(s_v2, si + 1)
                for g in range(a, b):
                    if g >= nps:
                        tensor.wait_ge(s_po, g - nps + 1)
                    tiles = (
                        [lo_off[g] + t for t in range(Tj[g][0])]
                        + [T_lo_s[si] + hi_off[g] + t for t in range(Tj[g][1])]
                    )
                    for i, t in enumerate(tiles):
                        mm = tensor.matmul(
                            ps[g % nps][:],
                            mbuf[si % 2][:, :, t],
                            msg[si % 2][:, t, :],
                            start=(i == 0), stop=(i == len(tiles) - 1),
                        )
                    mm.then_inc(s_mm, 1)

        @blockC.scalar
        def _(scalar):
            for g in range(G):
                scalar.wait_ge(s_mm, g + 1)
                scalar.copy(out_sb[:, g, :], ps[g % nps][:]).then_inc(s_po, 1)

        @blockC.sync
        def _(sync):
            phaseA_sync(sync)
            ostep = (G + 7) // 8
            nod = 0
            for a in range(0, G, ostep):
                b = min(G, a + ostep)
                sync.wait_ge(s_po, b)
                sync.dma_start(
                    out=out_d[:, a * D:b * D], in_=out_sb[:, a:b, :]
                ).then_inc(s_od, 16)
                nod += 16
            sync.wait_ge(s_od, nod)

    nc.compile()
    return nc


def _assemble(cfg, plan, outs):
    D, G, NS = cfg.D, cfg.G, cfg.NS
    full = np.empty((cfg.N, D), np.float32)
    for c in range(cfg.C):
        o = outs[c]["out"].reshape(P, G, D).transpose(1, 0, 2).reshape(
            cfg.NS_PAD, D)
        full[c * NS:(c + 1) * NS] = o[:NS]
    return full


def gnn_kernel(x, edge_src, edge_dst, edge_weight, w_bases, w_rel,
               cfg=None, trace=False):
    if cfg is None:
        cfg = Cfg(N=50000, E=800000)
    plan, in_maps = plan_and_pack(cfg, np.asarray(x), np.asarray(edge_src),
                                  np.asarray(edge_dst), np.asarray(edge_weight),
                                  np.asarray(w_bases), np.asarray(w_rel))
    nc = build_nc(cfg, plan)
    res = run_bass_kernel_spmd(nc, in_maps, list(range(cfg.C)), trace=trace)
    return _assemble(cfg, plan, res.results), res


def kernel(x, edge_src, edge_dst, edge_weight, w_bases, w_rel):
    """Full inputs in, full output out. Shards across 8 NeuronCores inside."""
    cfg = Cfg(N=50000, E=800000)
    plan, in_maps = plan_and_pack(cfg, np.asarray(x), np.asarray(edge_src),
                                  np.asarray(edge_dst), np.asarray(edge_weight),
                                  np.asarray(w_bases), np.asarray(w_rel))
    nc = build_nc(cfg, plan)
    res = run_bass_kernel_spmd(nc, in_maps, list(range(cfg.C)))
    return _assemble(cfg, plan, res.results)
